# revision 37
# baseline (speedup 1.0000x reference)
"""DeformableConv2D (DCNv2) forward on 8 Trainium2 NeuronCores.

Data-parallel over batch: one sample per core. Per core: offset conv on the
tensor engine (fp16 operands, fp32 accumulate, bias folded into the scalar-
engine PSUM copy); sampling coordinates and bilinear weights on the vector
engine (sigmoid via odd polynomial); gather indices wrapped into the SWDGE
16-partition layout with a pair of PE transposes (no DRAM roundtrip);
modulated bilinear sampling via one SWDGE dma_gather of 2x2-patch rows per
128-pixel block; corner combination via packed-fp16 broadcast multiply (DVE
2x mode) + accumulating PE transposes; im2col GEMM with the columns
stationary and the filter moving (FD=256), output stored pixel-major per
block. The front end is split into seven chunks (2/2/4/6/6/6/6 blocks)
pipelined against the gather stream.
"""
import sys
sys.path.insert(0, "/opt/trn_rl_repo")

import numpy as np
import ml_dtypes

import concourse.bass as bass
import concourse.bacc as bacc
import concourse.mybir as mybir
import concourse.tile as tile
from concourse import library_config

F32 = mybir.dt.float32
F16 = mybir.dt.float16
I16 = mybir.dt.int16
AL = mybir.AluOpType
AF = mybir.ActivationFunctionType

H = W = 64
C = 128
F = 256
K = 9
PADR = 8                 # padded-coordinate margin
HP = WP = 80             # padded image
NPIX = H * W             # 4096
NBLK = 32                # pixel blocks of 128 (2 rows each)
CONVW = 66               # conv grid width (pad 1)
XCLM = 67 + 9 * 512 + 67  # xcl with shift margins
NROWS = 2 * HP * 40      # pair-table rows = 6400
CHUNKS = [(0, 2), (2, 4), (4, 8), (8, 14), (14, 20), (20, 26), (26, 32)]

# sigmoid(x) ~= 0.5 + x*(C1 + C3 z + C5 z^2 + C7 z^3), z = x^2, |x| <= 2.75
SB = 2.75
SC1, SC3, SC5, SC7 = 0.24955315, -0.019879351, 1.5030454e-3, -5.8584555e-5

DY = np.repeat(np.arange(3) - 1, 3).astype(np.float32)   # per-tap dy
DX = np.tile(np.arange(3) - 1, 3).astype(np.float32)     # per-tap dx


def bcast(ap, shape):
    return ap.to_broadcast(list(shape))


_NC = None


def build_nc():
    nc = bacc.Bacc("TRN2", target_bir_lowering=False,
                   dynamic_dma_scratch_size=40960)
    xcl = nc.dram_tensor("xcl", [C, XCLM], F16, kind="ExternalInput")
    pairs = nc.dram_tensor("pairs", [NROWS, 512], F16, kind="ExternalInput")
    offk = nc.dram_tensor("offk", [C, K * 27], F16, kind="ExternalInput")
    offb = nc.dram_tensor("offb", [27, 1], F32, kind="ExternalInput")
    filt = nc.dram_tensor("filt", [C, K * F], F16, kind="ExternalInput")
    eye16 = nc.dram_tensor("eye16", [128, 128], F16, kind="ExternalInput")
    eye32 = nc.dram_tensor("eye32", [128, 128], F32, kind="ExternalInput")
    rep16 = nc.dram_tensor("rep16", [16, 128], F32, kind="ExternalInput")
    # consts: Y_all [128,32], dy/dx rows [128,9] each, X_all [128,1]
    consts = nc.dram_tensor("consts", [128, 348], F32, kind="ExternalInput")
    out_d = nc.dram_tensor("out", [128, NBLK, F], F16, kind="ExternalOutput")

    with tile.TileContext(nc) as tc:
        with (
            tc.tile_pool(name="const", bufs=1) as cpool,
            tc.tile_pool(name="front", bufs=2) as fpool,
            tc.tile_pool(name="cv", bufs=3) as cvpool,
            tc.tile_pool(name="wp", bufs=3) as wpool,
            tc.tile_pool(name="convps", bufs=1, space="PSUM") as convpool,
            tc.tile_pool(name="fps", bufs=2, space="PSUM") as fpspool,
            tc.tile_pool(name="pc0", bufs=2, space="PSUM") as pc0pool,
            tc.tile_pool(name="pc1", bufs=1, space="PSUM") as pc1pool,
            tc.tile_pool(name="pc2", bufs=1, space="PSUM") as pc2pool,
            tc.tile_pool(name="po", bufs=1, space="PSUM") as popool,
            tc.tile_pool(name="sg", bufs=3) as sgpool,
            tc.tile_pool(name="gw", bufs=4) as gwpool,
            tc.tile_pool(name="we8", bufs=8) as we8pool,
            tc.tile_pool(name="blk", bufs=2) as blkpool,
        ):
            nc.gpsimd.load_library(library_config.mlp)

            # preload the sigmoid act-function table off the critical path
            scr = cpool.tile([1, 1], F32)
            nc.vector.memset(scr[:], 0.0)
            nc.scalar.activation(out=scr[:], in_=scr[:], func=AF.Sigmoid)

            s_xcl = cpool.tile([C, XCLM], F16)
            nc.sync.dma_start(out=s_xcl[:, 0:800], in_=xcl[:, 0:800])
            s_offk = cpool.tile([C, K * 27], F16)
            nc.sync.dma_start(out=s_offk[:], in_=offk[:])
            s_offb = cpool.tile([27, 1], F32)
            nc.sync.dma_start(out=s_offb[:], in_=offb[:])
            s_const = cpool.tile([128, 348], F32)
            nc.sync.dma_start(out=s_const[:], in_=consts[:])
            s_eye32 = cpool.tile([128, 128], F32)
            nc.sync.dma_start(out=s_eye32[:], in_=eye32[:])
            s_eye16 = cpool.tile([128, 128], F16)
            nc.sync.dma_start(out=s_eye16[:], in_=eye16[:])
            s_rep16 = cpool.tile([16, 128], F32)
            nc.sync.dma_start(out=s_rep16[:], in_=rep16[:])
            nc.sync.dma_start(out=s_xcl[:, 800:2200], in_=xcl[:, 800:2200])
            nc.sync.dma_start(out=s_xcl[:, 2200:XCLM], in_=xcl[:, 2200:XCLM])
            s_filt = cpool.tile([C, K * F], F16)
            nc.sync.dma_start(out=s_filt[:], in_=filt[:])
            # wrapped gather indices for all 32 blocks, zero-initialized off
            # the DVE (scalar-engine memzero)
            idxw_all = cpool.tile([128, NBLK * 72], I16)
            nc.scalar.memzero(idxw_all[:])

            y_all = s_const[:, 0:32]          # [128, 32]
            x_all = s_const[:, 50:51]         # [128, 1]
            yc_t = s_const[:, 51:339].rearrange("p (b k) -> p b k", k=9)
            xc_t = s_const[:, 339:348]        # [128, 9] = x + dx + 8

            w16_c = [None] * len(CHUNKS)      # per-chunk weights [128,n,36] f16
            wiT_c = [None] * len(CHUNKS)      # per-chunk conv outputs
            we8_c = {}                        # per-block broadcast weights
            dst_t = {}                        # per-block gather destinations

            from contextlib import contextmanager

            @contextmanager
            def prio(base):
                orig = tc.cur_priority
                tc.cur_priority = base
                try:
                    yield
                finally:
                    tc.cur_priority = orig

            def conv_chunk(ci):
                """Offset conv for blocks [lo, hi) -> wiT_c[ci]. PE/Act only;
                runs just ahead of the matching stage-B chunk."""
                with prio(-1010000 + ci * 2000):
                    _conv_chunk(ci)

            def stageb_chunk(ci):
                """Coordinate/weight math (DVE) + wrapped-index staging for
                blocks [lo, hi); fills w16_c and idxw_all columns."""
                with prio(-1009000 + ci * 2000):
                    _stageb_chunk(ci)

            def _conv_chunk(ci):
                lo, hi = CHUNKS[ci]
                n = hi - lo
                wi_c = cvpool.tile([27, 6, 128], F32, tag="wic")
                wiT = cvpool.tile([128, 6, 27], F32, tag="wiT")
                wiT_c[ci] = wiT
                for bi in range(0, n, 2):
                    b = lo + bi
                    q0 = (2 * b + 1) * CONVW
                    ps = convpool.tile([27, 264], F32, tag="convps")
                    for t in range(K):
                        d = int(DY[t]) * CONVW + int(DX[t])
                        nc.tensor.matmul(
                            out=ps[:],
                            lhsT=s_offk[:, t * 27:(t + 1) * 27],
                            rhs=s_xcl[:, 67 + q0 + d: 67 + q0 + 264 + d],
                            start=(t == 0), stop=(t == K - 1),
                        )
                    # bias folded into the PSUM->SBUF copy (per-partition bias)
                    nc.scalar.activation(
                        out=wi_c[:, bi:bi + 2, :].rearrange(
                            "p b (r x) -> p (b r) x", x=64),
                        in_=ps[:].rearrange("p (r x) -> p r x", x=CONVW)[:, :, 1:65],
                        func=AF.Identity, bias=s_offb[:, 0:1])
                    for u in range(2):
                        pt = fpspool.tile([128, 27], F32, tag="fps")
                        nc.tensor.transpose(
                            out=pt[:], in_=wi_c[:, bi + u, :],
                            identity=s_eye32[:27, :27])
                        nc.scalar.copy(out=wiT[:, bi + u, :], in_=pt[:])

            def _stageb_chunk(ci):
                lo, hi = CHUNKS[ci]
                n = hi - lo
                G = 9 * n
                wiT = wiT_c[ci]
                o1 = wiT[:, 0:n, 0:9]
                o2 = wiT[:, 0:n, 9:18]
                mm = wiT[:, 0:n, 18:27]
                S = [128, n, 9]
                yh = y_all[:, lo:hi]

                def ftile(tag):
                    return fpool.tile([128, 6, 9], F32, tag=tag,
                                      name=f"{tag}{ci}")[:, 0:n, :]

                # sigmoid on the scalar engine's activation table (off DVE)
                sigm = ftile("sigm")
                nc.scalar.activation(out=sigm[:], in_=mm, func=AF.Sigmoid)

                py = ftile("py")
                nc.vector.tensor_tensor(out=py[:], in0=o1,
                                        in1=yc_t[:, lo:hi, :], op=AL.add)
                nc.vector.tensor_scalar(out=py[:], in0=py[:], scalar1=2.0,
                                        scalar2=77.0, op0=AL.max, op1=AL.min)
                y0p = ftile("y0p")
                nc.vector.tensor_scalar(out=y0p[:], in0=py[:], scalar1=-0.5,
                                        scalar2=8388608.0, op0=AL.add, op1=AL.add)
                nc.vector.tensor_scalar(out=y0p[:], in0=y0p[:], scalar1=-8388608.0,
                                        scalar2=None, op0=AL.add)
                fy = ftile("fy")
                nc.vector.tensor_tensor(out=fy[:], in0=py[:], in1=y0p[:],
                                        op=AL.subtract)
                wy0 = ftile("wy0")
                nc.vector.tensor_scalar(out=wy0[:], in0=fy[:], scalar1=-1.0,
                                        scalar2=1.0, op0=AL.mult, op1=AL.add)

                px = ftile("px")
                nc.vector.tensor_tensor(
                    out=px[:], in0=o2,
                    in1=bcast(xc_t.rearrange("p (o k) -> p o k", o=1), S),
                    op=AL.add)
                nc.vector.tensor_scalar(out=px[:], in0=px[:], scalar1=2.0,
                                        scalar2=77.0, op0=AL.max, op1=AL.min)
                x0p = ftile("x0p")
                nc.vector.tensor_scalar(out=x0p[:], in0=px[:], scalar1=-0.5,
                                        scalar2=8388608.0, op0=AL.add, op1=AL.add)
                nc.vector.tensor_scalar(out=x0p[:], in0=x0p[:], scalar1=-8388608.0,
                                        scalar2=None, op0=AL.add)
                fx = ftile("fx")
                nc.vector.tensor_tensor(out=fx[:], in0=px[:], in1=x0p[:],
                                        op=AL.subtract)
                wx0 = ftile("wx0")
                nc.vector.tensor_scalar(out=wx0[:], in0=fx[:], scalar1=-1.0,
                                        scalar2=1.0, op0=AL.mult, op1=AL.add)

                qx = ftile("qx")
                nc.vector.tensor_scalar(out=qx[:], in0=x0p[:], scalar1=0.5,
                                        scalar2=-0.25, op0=AL.mult, op1=AL.add)
                nc.vector.tensor_scalar(out=qx[:], in0=qx[:], scalar1=8388608.0,
                                        scalar2=-8388608.0, op0=AL.add, op1=AL.add)
                parx = ftile("parx")
                nc.vector.scalar_tensor_tensor(
                    out=parx[:], in0=qx[:], scalar=-2.0, in1=x0p[:],
                    op0=AL.mult, op1=AL.add)
                qy = ftile("qy")
                nc.vector.tensor_scalar(out=qy[:], in0=y0p[:], scalar1=0.5,
                                        scalar2=-0.25, op0=AL.mult, op1=AL.add)
                nc.vector.tensor_scalar(out=qy[:], in0=qy[:], scalar1=8388608.0,
                                        scalar2=-8388608.0, op0=AL.add, op1=AL.add)
                pary = ftile("pary")
                nc.vector.scalar_tensor_tensor(
                    out=pary[:], in0=qy[:], scalar=-2.0, in1=y0p[:],
                    op0=AL.mult, op1=AL.add)
                base = ftile("base")
                nc.vector.scalar_tensor_tensor(
                    out=base[:], in0=qy[:], scalar=40.0, in1=qx[:],
                    op0=AL.mult, op1=AL.add)
                nc.vector.scalar_tensor_tensor(
                    out=base[:], in0=parx[:], scalar=1600.0, in1=base[:],
                    op0=AL.mult, op1=AL.add)
                nc.vector.scalar_tensor_tensor(
                    out=base[:], in0=pary[:], scalar=3200.0, in1=base[:],
                    op0=AL.mult, op1=AL.add)

                # wrapped-index staging fully on-chip: double PE transpose
                # lands base[pg*16+pp, g] on partition pp at (pg, g) — the
                # SWDGE 16-partition-wrapped layout — then a matmul against a
                # constant 16->128 replication matrix duplicates the wrapped
                # block into all 8 partition groups (HW SWDGE reads them all).
                tg_ps = fpspool.tile([G, 128], F32, tag="fps", name=f"tg{ci}")
                nc.tensor.transpose(
                    out=tg_ps[:],
                    in_=base[:].rearrange("p b k -> p (b k)"),
                    identity=s_eye32[:])
                tgs = fpool.tile([54, 128], F32, tag="tgs")
                nc.scalar.copy(out=tgs[0:G, :], in_=tg_ps[:])
                p2 = fpspool.tile([16, 8 * G], F32, tag="fps", name=f"p2{ci}")
                for pg in range(8):
                    nc.tensor.transpose(
                        out=p2[:, pg * G:(pg + 1) * G],
                        in_=tgs[0:G, pg * 16:(pg + 1) * 16],
                        identity=s_eye32[:G, :G])
                s2 = fpool.tile([16, 8 * 54], F32, tag="s2")
                nc.scalar.copy(out=s2[:, 0:8 * G], in_=p2[:])
                rep_ps = fpspool.tile([128, 8 * G], F32, tag="fps",
                                      name=f"rp{ci}")
                nc.tensor.matmul(
                    out=rep_ps[:],
                    lhsT=s_rep16[:],
                    rhs=s2[:, 0:8 * G],
                    start=True, stop=True)

                # weights W [128, n, 9, 2, 2]  (k, yc, xc) — issued between
                # the PE index chain and its DVE copy-out so the DVE queue
                # stays fed while the transposes run
                a0 = ftile("a0")
                nc.vector.tensor_tensor(out=a0[:], in0=wy0[:], in1=sigm[:],
                                        op=AL.mult)
                a1 = ftile("a1")
                nc.vector.tensor_tensor(out=a1[:], in0=fy[:], in1=sigm[:],
                                        op=AL.mult)
                w_f32 = fpool.tile([128, 6, 9, 2, 2], F32, tag="wf")
                nc.vector.tensor_tensor(out=w_f32[:, 0:n, :, 0, 0], in0=a0[:],
                                        in1=wx0[:], op=AL.mult)
                nc.vector.tensor_tensor(out=w_f32[:, 0:n, :, 0, 1], in0=a0[:],
                                        in1=fx[:], op=AL.mult)
                nc.vector.tensor_tensor(out=w_f32[:, 0:n, :, 1, 0], in0=a1[:],
                                        in1=wx0[:], op=AL.mult)
                nc.vector.tensor_tensor(out=w_f32[:, 0:n, :, 1, 1], in0=a1[:],
                                        in1=fx[:], op=AL.mult)
                w16 = wpool.tile([128, 6, 36], F16, tag="w16", name=f"w16{ci}")
                nc.vector.tensor_copy(
                    out=w16[:, 0:n, :],
                    in_=w_f32[:, 0:n].rearrange("p b k y u -> p b (k y u)"))
                w16_c[ci] = w16
                nc.vector.tensor_copy(
                    out=idxw_all[:, lo * 72:hi * 72].rearrange(
                        "p (b k pg) -> p b k pg", k=9, pg=8),
                    in_=rep_ps[:].rearrange("p (pg b k) -> p b k pg", b=n, k=9))

            def emit_we8(b):
                """Broadcast per-block weights for the DVE 16x8 multiply
                pattern. Depends only on w16; emitted a couple of blocks ahead
                of its consumer so the gwv chain never waits on the scalar
                queue."""
                ci, lo = chunk_of_block(b)
                w16 = w16_c[ci]
                w_e8 = we8pool.tile([128, 36, 8], F16, tag="we8",
                                    name=f"we8_{b}")
                nc.scalar.copy(
                    out=w_e8[:],
                    in_=bcast(
                        w16[:, b - lo, :].rearrange("p (j o) -> p j o", o=1),
                        [128, 36, 8]))
                we8_c[b] = w_e8

            def chunk_of_block(b):
                for ci, (lo, hi) in enumerate(CHUNKS):
                    if lo <= b < hi:
                        return ci, lo
                raise AssertionError

            def emit_gather(b):
                with prio(-900000 + b * 2000):
                    dstb = sgpool.tile([128, K, 512], F16, tag="dst")
                    nc.gpsimd.dma_gather(
                        dstb[:, 0:5, :], pairs[:],
                        idxw_all[:, b * 72:b * 72 + 40],
                        5 * 128, 5 * 128, 512)
                    nc.gpsimd.dma_gather(
                        dstb[:, 5:K, :], pairs[:],
                        idxw_all[:, b * 72 + 40:(b + 1) * 72],
                        4 * 128, 4 * 128, 512)
                    dst_t[b] = dstb

            def compute_blk(b):
                dstb = dst_t.pop(b)
                w_e8 = we8_c.pop(b)
                ci, lo = chunk_of_block(b)
                # interleave with the front: this block's work sorts just
                # after the NEXT chunk's stage-B, so compute never waits for
                # the whole front band and each stage-B stays one chunk ahead
                # of its gathers
                with prio(-1009000 + (ci + 1) * 2000 + 500 + (b - lo) * 10):
                    if b + 2 < NBLK and b + 2 >= 4:
                        emit_we8(b + 2)
                    gw = gwpool.tile([128, 36, 128], F16, tag="gw")
                    dsrc = dstb[:].rearrange("p s e -> p (s e)").rearrange(
                        "p (j r q) -> p j r q", r=16, q=8)
                    gwv = gw[:].rearrange("p j (r q) -> p j r q", q=8)
                    w_in = bcast(w_e8[:].rearrange("p j (o q) -> p j o q", o=1),
                                 [128, 36, 16, 8])
                    # packed fp16 operands -> DVE 2x mode
                    nc.vector.tensor_tensor(out=gwv[:], in0=dsrc[:],
                                            in1=w_in[:], op=AL.mult)
                    cols = blkpool.tile([128, K, 128], F16, tag="colsb")
                    # accumulating transposes: 4 taps per PSUM bank
                    for kg, pool, nk in ((0, pc0pool, 4), (1, pc1pool, 4),
                                         (2, pc2pool, 1)):
                        pcb = pool.tile([128, nk * 128], F32, tag=f"pc{kg}")
                        for kq in range(nk):
                            k = 4 * kg + kq
                            for j in range(4):
                                nc.tensor.matmul(
                                    out=pcb[:, kq * 128:(kq + 1) * 128],
                                    lhsT=gw[:, 4 * k + j, :],
                                    rhs=s_eye16[:], start=(j == 0), stop=(j == 3))
                        nc.scalar.copy(
                            out=cols[:, 4 * kg:4 * kg + nk, :],
                            in_=pcb[:].rearrange("p (k c) -> p k c", c=128))
                    # im2col GEMM: cols stationary, filter moving (FD=256),
                    # output pixel-major [pix, F]
                    po = popool.tile([128, F], F32, tag="po")
                    for k in range(K):
                        nc.tensor.matmul(
                            out=po[:],
                            lhsT=cols[:, k, :],
                            rhs=s_filt[:, k * F:(k + 1) * F],
                            start=(k == 0), stop=(k == K - 1))
                    osb = blkpool.tile([128, F], F16, tag="osb")
                    nc.scalar.copy(out=osb[:], in_=po[:])
                    nc.sync.dma_start(out=out_d[:, b, :], in_=osb[:])

            for ci in range(len(CHUNKS)):
                conv_chunk(ci)
                stageb_chunk(ci)
                if ci < 2:
                    with prio(-1009000 + ci * 2000 + 500):
                        for b in range(*CHUNKS[ci]):
                            emit_we8(b)
                for b in range(*CHUNKS[ci]):
                    emit_gather(b)
            for b in range(NBLK):
                compute_blk(b)
    nc.compile()
    return nc


def host_inputs(x, offset_kernel, offset_bias, filt_w):
    """Per-sample input maps. x [8,64,64,128] f32 etc (numpy)."""
    offk = np.ascontiguousarray(
        offset_kernel.reshape(K, C, 27).transpose(1, 0, 2).reshape(C, K * 27)
    ).astype(np.float16)
    offb = offset_bias.reshape(27, 1).astype(np.float32)
    filt_re = np.ascontiguousarray(
        filt_w.reshape(K, C, F).transpose(1, 0, 2).reshape(C, K * F)
    ).astype(np.float16)
    eye16 = np.eye(128).astype(np.float16)
    eye32 = np.eye(128).astype(np.float32)
    rep16 = np.zeros((16, 128), np.float32)
    rep16[np.arange(128) % 16, np.arange(128)] = 1.0
    consts = np.zeros((128, 348), np.float32)
    p = np.arange(128)
    yoff = p // 64
    consts[:, 0:32] = 2 * np.arange(32)[None, :] + yoff[:, None]
    consts[:, 32:41] = DY[None, :]
    consts[:, 41:50] = DX[None, :]
    consts[:, 50] = p % 64
    consts[:, 51:339] = (consts[:, 0:32, None] + DY[None, None, :]
                         + 8.0).reshape(128, 288)
    consts[:, 339:348] = consts[:, 50:51] + DX[None, :] + 8.0

    maps = []
    for b in range(x.shape[0]):
        xp = np.zeros((HP + 2, WP + 2, C), np.float32)
        xp[PADR:PADR + H, PADR:PADR + W] = x[b]
        quad = np.zeros((2, 2, 40, 40, 2, 2, C), np.float32)
        for pY in range(2):
            for pX in range(2):
                for uy in range(2):
                    for ux in range(2):
                        quad[pY, pX, :, :, uy, ux] = \
                            xp[pY + uy:pY + uy + 80:2, pX + ux:pX + ux + 80:2]
        prs = quad.reshape(NROWS, 4 * C).astype(np.float16)

        x1 = np.zeros((CONVW, CONVW, C), np.float32)
        x1[1:65, 1:65] = x[b]
        xcl = np.zeros((C, XCLM), np.float16)
        xcl[:, 67:67 + 4356] = x1.reshape(CONVW * CONVW, C).T.astype(np.float16)
        maps.append({
            "xcl": xcl, "pairs": prs, "offk": offk, "offb": offb,
            "filt": filt_re, "eye16": eye16, "eye32": eye32, "rep16": rep16,
            "consts": consts,
        })
    return maps


def host_output(res_list):
    outs = []
    for r in res_list:
        o = r["out"].astype(np.float32).reshape(128, NBLK, F)
        outs.append(np.ascontiguousarray(
            o.transpose(1, 0, 2)).reshape(H, W, F))
    return np.stack(outs)


def _get_nc():
    global _NC
    if _NC is None:
        _NC = build_nc()
    return _NC


def kernel(inputs, offset_kernel, offset_bias, filt):
    from concourse.bass_utils import run_bass_kernel_spmd
    x = np.asarray(inputs, dtype=np.float32)
    maps = host_inputs(x, np.asarray(offset_kernel, np.float32),
                       np.asarray(offset_bias, np.float32),
                       np.asarray(filt, np.float32))
    nc = _get_nc()
    res = run_bass_kernel_spmd(nc, maps, core_ids=list(range(8)))
    return host_output(res.results).astype(np.float32)


# revision 38
# speedup vs baseline: 1.0280x; 1.0280x over previous
"""DeformableConv2D (DCNv2) forward on 8 Trainium2 NeuronCores.

Data-parallel over batch: one sample per core. Per core: offset conv on the
tensor engine (fp16 operands, fp32 accumulate, bias folded into the scalar-
engine PSUM copy); sampling coordinates and bilinear weights on the vector
engine (sigmoid via odd polynomial); gather indices wrapped into the SWDGE
16-partition layout with a pair of PE transposes (no DRAM roundtrip);
modulated bilinear sampling via one SWDGE dma_gather of 2x2-patch rows per
128-pixel block; corner combination via packed-fp16 broadcast multiply (DVE
2x mode) + accumulating PE transposes; im2col GEMM with the columns
stationary and the filter moving (FD=256), output stored pixel-major per
block. The front end is split into seven chunks (2/2/4/6/6/6/6 blocks)
pipelined against the gather stream.
"""
import sys
sys.path.insert(0, "/opt/trn_rl_repo")

import numpy as np
import ml_dtypes

import concourse.bass as bass
import concourse.bacc as bacc
import concourse.mybir as mybir
import concourse.tile as tile
from concourse import library_config

F32 = mybir.dt.float32
F16 = mybir.dt.float16
I16 = mybir.dt.int16
AL = mybir.AluOpType
AF = mybir.ActivationFunctionType

H = W = 64
C = 128
F = 256
K = 9
PADR = 8                 # padded-coordinate margin
HP = WP = 80             # padded image
NPIX = H * W             # 4096
NBLK = 32                # pixel blocks of 128 (2 rows each)
CONVW = 66               # conv grid width (pad 1)
XCLM = 67 + 9 * 512 + 67  # xcl with shift margins
NROWS = 2 * HP * 40      # pair-table rows = 6400
CHUNKS = [(0, 2), (2, 4), (4, 8), (8, 14), (14, 20), (20, 26), (26, 32)]

# sigmoid(x) ~= 0.5 + x*(C1 + C3 z + C5 z^2 + C7 z^3), z = x^2, |x| <= 2.75
SB = 2.75
SC1, SC3, SC5, SC7 = 0.24955315, -0.019879351, 1.5030454e-3, -5.8584555e-5

DY = np.repeat(np.arange(3) - 1, 3).astype(np.float32)   # per-tap dy
DX = np.tile(np.arange(3) - 1, 3).astype(np.float32)     # per-tap dx


def bcast(ap, shape):
    return ap.to_broadcast(list(shape))


_NC = None


def build_nc():
    nc = bacc.Bacc("TRN2", target_bir_lowering=False,
                   dynamic_dma_scratch_size=40960)
    xcl = nc.dram_tensor("xcl", [C, XCLM], F16, kind="ExternalInput")
    pairs = nc.dram_tensor("pairs", [NROWS, 512], F16, kind="ExternalInput")
    offk = nc.dram_tensor("offk", [C, K * 27], F16, kind="ExternalInput")
    offb = nc.dram_tensor("offb", [27, 1], F32, kind="ExternalInput")
    filt = nc.dram_tensor("filt", [C, K * F], F16, kind="ExternalInput")
    eye16 = nc.dram_tensor("eye16", [128, 128], F16, kind="ExternalInput")
    eye32 = nc.dram_tensor("eye32", [128, 128], F32, kind="ExternalInput")
    rep16 = nc.dram_tensor("rep16", [16, 128], F32, kind="ExternalInput")
    # consts: Y_all [128,32], dy/dx rows [128,9] each, X_all [128,1]
    consts = nc.dram_tensor("consts", [128, 348], F32, kind="ExternalInput")
    out_d = nc.dram_tensor("out", [128, NBLK, F], F16, kind="ExternalOutput")

    with tile.TileContext(nc) as tc:
        with (
            tc.tile_pool(name="const", bufs=1) as cpool,
            tc.tile_pool(name="front", bufs=2) as fpool,
            tc.tile_pool(name="cv", bufs=3) as cvpool,
            tc.tile_pool(name="wp", bufs=3) as wpool,
            tc.tile_pool(name="convps", bufs=1, space="PSUM") as convpool,
            tc.tile_pool(name="fps", bufs=2, space="PSUM") as fpspool,
            tc.tile_pool(name="pc0", bufs=2, space="PSUM") as pc0pool,
            tc.tile_pool(name="pc1", bufs=1, space="PSUM") as pc1pool,
            tc.tile_pool(name="pc2", bufs=1, space="PSUM") as pc2pool,
            tc.tile_pool(name="po", bufs=1, space="PSUM") as popool,
            tc.tile_pool(name="sg", bufs=3) as sgpool,
            tc.tile_pool(name="gw", bufs=6) as gwpool,
            tc.tile_pool(name="we8", bufs=8) as we8pool,
            tc.tile_pool(name="blk", bufs=2) as blkpool,
        ):
            nc.gpsimd.load_library(library_config.mlp)

            # preload the sigmoid act-function table off the critical path
            scr = cpool.tile([1, 1], F32)
            nc.vector.memset(scr[:], 0.0)
            nc.scalar.activation(out=scr[:], in_=scr[:], func=AF.Sigmoid)

            s_xcl = cpool.tile([C, XCLM], F16)
            nc.sync.dma_start(out=s_xcl[:, 0:800], in_=xcl[:, 0:800])
            s_offk = cpool.tile([C, K * 27], F16)
            nc.sync.dma_start(out=s_offk[:], in_=offk[:])
            s_offb = cpool.tile([27, 1], F32)
            nc.sync.dma_start(out=s_offb[:], in_=offb[:])
            s_const = cpool.tile([128, 348], F32)
            nc.sync.dma_start(out=s_const[:], in_=consts[:])
            s_eye32 = cpool.tile([128, 128], F32)
            nc.sync.dma_start(out=s_eye32[:], in_=eye32[:])
            s_eye16 = cpool.tile([128, 128], F16)
            nc.sync.dma_start(out=s_eye16[:], in_=eye16[:])
            s_rep16 = cpool.tile([16, 128], F32)
            nc.sync.dma_start(out=s_rep16[:], in_=rep16[:])
            nc.sync.dma_start(out=s_xcl[:, 800:2200], in_=xcl[:, 800:2200])
            nc.sync.dma_start(out=s_xcl[:, 2200:XCLM], in_=xcl[:, 2200:XCLM])
            s_filt = cpool.tile([C, K * F], F16)
            nc.sync.dma_start(out=s_filt[:], in_=filt[:])
            # wrapped gather indices for all 32 blocks, zero-initialized off
            # the DVE (scalar-engine memzero)
            idxw_all = cpool.tile([128, NBLK * 72], I16)
            nc.scalar.memzero(idxw_all[:])

            y_all = s_const[:, 0:32]          # [128, 32]
            x_all = s_const[:, 50:51]         # [128, 1]
            yc_t = s_const[:, 51:339].rearrange("p (b k) -> p b k", k=9)
            xc_t = s_const[:, 339:348]        # [128, 9] = x + dx + 8

            w16_c = [None] * len(CHUNKS)      # per-chunk weights [128,n,36] f16
            wiT_c = [None] * len(CHUNKS)      # per-chunk conv outputs
            we8_c = {}                        # per-block broadcast weights
            dst_t = {}                        # per-block gather destinations

            from contextlib import contextmanager

            @contextmanager
            def prio(base):
                orig = tc.cur_priority
                tc.cur_priority = base
                try:
                    yield
                finally:
                    tc.cur_priority = orig

            def conv_chunk(ci):
                """Offset conv for blocks [lo, hi) -> wiT_c[ci]. PE/Act only;
                runs just ahead of the matching stage-B chunk."""
                with prio(-1010000 + ci * 2000):
                    _conv_chunk(ci)

            def stageb_chunk(ci):
                """Coordinate/weight math (DVE) + wrapped-index staging for
                blocks [lo, hi); fills w16_c and idxw_all columns."""
                with prio(-1009000 + ci * 2000):
                    _stageb_chunk(ci)

            def _conv_chunk(ci):
                lo, hi = CHUNKS[ci]
                n = hi - lo
                wi_c = cvpool.tile([27, 6, 128], F32, tag="wic")
                wiT = cvpool.tile([128, 6, 27], F32, tag="wiT")
                wiT_c[ci] = wiT
                for bi in range(0, n, 2):
                    b = lo + bi
                    q0 = (2 * b + 1) * CONVW
                    ps = convpool.tile([27, 264], F32, tag="convps")
                    for t in range(K):
                        d = int(DY[t]) * CONVW + int(DX[t])
                        nc.tensor.matmul(
                            out=ps[:],
                            lhsT=s_offk[:, t * 27:(t + 1) * 27],
                            rhs=s_xcl[:, 67 + q0 + d: 67 + q0 + 264 + d],
                            start=(t == 0), stop=(t == K - 1),
                        )
                    # bias folded into the PSUM->SBUF copy (per-partition bias)
                    nc.scalar.activation(
                        out=wi_c[:, bi:bi + 2, :].rearrange(
                            "p b (r x) -> p (b r) x", x=64),
                        in_=ps[:].rearrange("p (r x) -> p r x", x=CONVW)[:, :, 1:65],
                        func=AF.Identity, bias=s_offb[:, 0:1])
                    for u in range(2):
                        pt = fpspool.tile([128, 27], F32, tag="fps")
                        nc.tensor.transpose(
                            out=pt[:], in_=wi_c[:, bi + u, :],
                            identity=s_eye32[:27, :27])
                        nc.scalar.copy(out=wiT[:, bi + u, :], in_=pt[:])

            def _stageb_chunk(ci):
                lo, hi = CHUNKS[ci]
                n = hi - lo
                G = 9 * n
                wiT = wiT_c[ci]
                o1 = wiT[:, 0:n, 0:9]
                o2 = wiT[:, 0:n, 9:18]
                mm = wiT[:, 0:n, 18:27]
                S = [128, n, 9]
                yh = y_all[:, lo:hi]

                def ftile(tag):
                    return fpool.tile([128, 6, 9], F32, tag=tag,
                                      name=f"{tag}{ci}")[:, 0:n, :]

                # sigmoid on the scalar engine's activation table (off DVE)
                sigm = ftile("sigm")
                nc.scalar.activation(out=sigm[:], in_=mm, func=AF.Sigmoid)

                py = ftile("py")
                nc.vector.tensor_tensor(out=py[:], in0=o1,
                                        in1=yc_t[:, lo:hi, :], op=AL.add)
                nc.vector.tensor_scalar(out=py[:], in0=py[:], scalar1=2.0,
                                        scalar2=77.0, op0=AL.max, op1=AL.min)
                y0p = ftile("y0p")
                nc.vector.tensor_scalar(out=y0p[:], in0=py[:], scalar1=-0.5,
                                        scalar2=8388608.0, op0=AL.add, op1=AL.add)
                nc.vector.tensor_scalar(out=y0p[:], in0=y0p[:], scalar1=-8388608.0,
                                        scalar2=None, op0=AL.add)
                fy = ftile("fy")
                nc.vector.tensor_tensor(out=fy[:], in0=py[:], in1=y0p[:],
                                        op=AL.subtract)
                wy0 = ftile("wy0")
                nc.vector.tensor_scalar(out=wy0[:], in0=fy[:], scalar1=-1.0,
                                        scalar2=1.0, op0=AL.mult, op1=AL.add)

                px = ftile("px")
                nc.vector.tensor_tensor(
                    out=px[:], in0=o2,
                    in1=bcast(xc_t.rearrange("p (o k) -> p o k", o=1), S),
                    op=AL.add)
                nc.vector.tensor_scalar(out=px[:], in0=px[:], scalar1=2.0,
                                        scalar2=77.0, op0=AL.max, op1=AL.min)
                x0p = ftile("x0p")
                nc.vector.tensor_scalar(out=x0p[:], in0=px[:], scalar1=-0.5,
                                        scalar2=8388608.0, op0=AL.add, op1=AL.add)
                nc.vector.tensor_scalar(out=x0p[:], in0=x0p[:], scalar1=-8388608.0,
                                        scalar2=None, op0=AL.add)
                fx = ftile("fx")
                nc.vector.tensor_tensor(out=fx[:], in0=px[:], in1=x0p[:],
                                        op=AL.subtract)
                wx0 = ftile("wx0")
                nc.vector.tensor_scalar(out=wx0[:], in0=fx[:], scalar1=-1.0,
                                        scalar2=1.0, op0=AL.mult, op1=AL.add)

                qx = ftile("qx")
                nc.vector.tensor_scalar(out=qx[:], in0=x0p[:], scalar1=0.5,
                                        scalar2=-0.25, op0=AL.mult, op1=AL.add)
                nc.vector.tensor_scalar(out=qx[:], in0=qx[:], scalar1=8388608.0,
                                        scalar2=-8388608.0, op0=AL.add, op1=AL.add)
                parx = ftile("parx")
                nc.vector.scalar_tensor_tensor(
                    out=parx[:], in0=qx[:], scalar=-2.0, in1=x0p[:],
                    op0=AL.mult, op1=AL.add)
                qy = ftile("qy")
                nc.vector.tensor_scalar(out=qy[:], in0=y0p[:], scalar1=0.5,
                                        scalar2=-0.25, op0=AL.mult, op1=AL.add)
                nc.vector.tensor_scalar(out=qy[:], in0=qy[:], scalar1=8388608.0,
                                        scalar2=-8388608.0, op0=AL.add, op1=AL.add)
                pary = ftile("pary")
                nc.vector.scalar_tensor_tensor(
                    out=pary[:], in0=qy[:], scalar=-2.0, in1=y0p[:],
                    op0=AL.mult, op1=AL.add)
                base = ftile("base")
                nc.vector.scalar_tensor_tensor(
                    out=base[:], in0=qy[:], scalar=40.0, in1=qx[:],
                    op0=AL.mult, op1=AL.add)
                nc.vector.scalar_tensor_tensor(
                    out=base[:], in0=parx[:], scalar=1600.0, in1=base[:],
                    op0=AL.mult, op1=AL.add)
                nc.vector.scalar_tensor_tensor(
                    out=base[:], in0=pary[:], scalar=3200.0, in1=base[:],
                    op0=AL.mult, op1=AL.add)

                # wrapped-index staging fully on-chip: double PE transpose
                # lands base[pg*16+pp, g] on partition pp at (pg, g) — the
                # SWDGE 16-partition-wrapped layout — then a matmul against a
                # constant 16->128 replication matrix duplicates the wrapped
                # block into all 8 partition groups (HW SWDGE reads them all).
                tg_ps = fpspool.tile([G, 128], F32, tag="fps", name=f"tg{ci}")
                nc.tensor.transpose(
                    out=tg_ps[:],
                    in_=base[:].rearrange("p b k -> p (b k)"),
                    identity=s_eye32[:])
                tgs = fpool.tile([54, 128], F32, tag="tgs")
                nc.scalar.copy(out=tgs[0:G, :], in_=tg_ps[:])
                p2 = fpspool.tile([16, 8 * G], F32, tag="fps", name=f"p2{ci}")
                for pg in range(8):
                    nc.tensor.transpose(
                        out=p2[:, pg * G:(pg + 1) * G],
                        in_=tgs[0:G, pg * 16:(pg + 1) * 16],
                        identity=s_eye32[:G, :G])
                s2 = fpool.tile([16, 8 * 54], F32, tag="s2")
                nc.scalar.copy(out=s2[:, 0:8 * G], in_=p2[:])
                rep_ps = fpspool.tile([128, 8 * G], F32, tag="fps",
                                      name=f"rp{ci}")
                nc.tensor.matmul(
                    out=rep_ps[:],
                    lhsT=s_rep16[:],
                    rhs=s2[:, 0:8 * G],
                    start=True, stop=True)

                # weights W [128, n, 9, 2, 2]  (k, yc, xc) — issued between
                # the PE index chain and its DVE copy-out so the DVE queue
                # stays fed while the transposes run
                a0 = ftile("a0")
                nc.vector.tensor_tensor(out=a0[:], in0=wy0[:], in1=sigm[:],
                                        op=AL.mult)
                a1 = ftile("a1")
                nc.vector.tensor_tensor(out=a1[:], in0=fy[:], in1=sigm[:],
                                        op=AL.mult)
                w_f32 = fpool.tile([128, 6, 9, 2, 2], F32, tag="wf")
                nc.vector.tensor_tensor(out=w_f32[:, 0:n, :, 0, 0], in0=a0[:],
                                        in1=wx0[:], op=AL.mult)
                nc.vector.tensor_tensor(out=w_f32[:, 0:n, :, 0, 1], in0=a0[:],
                                        in1=fx[:], op=AL.mult)
                nc.vector.tensor_tensor(out=w_f32[:, 0:n, :, 1, 0], in0=a1[:],
                                        in1=wx0[:], op=AL.mult)
                nc.vector.tensor_tensor(out=w_f32[:, 0:n, :, 1, 1], in0=a1[:],
                                        in1=fx[:], op=AL.mult)
                w16 = wpool.tile([128, 6, 36], F16, tag="w16", name=f"w16{ci}")
                nc.vector.tensor_copy(
                    out=w16[:, 0:n, :],
                    in_=w_f32[:, 0:n].rearrange("p b k y u -> p b (k y u)"))
                w16_c[ci] = w16
                nc.vector.tensor_copy(
                    out=idxw_all[:, lo * 72:hi * 72].rearrange(
                        "p (b k pg) -> p b k pg", k=9, pg=8),
                    in_=rep_ps[:].rearrange("p (pg b k) -> p b k pg", b=n, k=9))

            def emit_we8(b):
                """Broadcast per-block weights for the DVE 16x8 multiply
                pattern. Depends only on w16; emitted a couple of blocks ahead
                of its consumer so the gwv chain never waits on the scalar
                queue."""
                ci, lo = chunk_of_block(b)
                w16 = w16_c[ci]
                w_e8 = we8pool.tile([128, 36, 8], F16, tag="we8",
                                    name=f"we8_{b}")
                nc.scalar.copy(
                    out=w_e8[:],
                    in_=bcast(
                        w16[:, b - lo, :].rearrange("p (j o) -> p j o", o=1),
                        [128, 36, 8]))
                we8_c[b] = w_e8

            def chunk_of_block(b):
                for ci, (lo, hi) in enumerate(CHUNKS):
                    if lo <= b < hi:
                        return ci, lo
                raise AssertionError

            def emit_gather(b):
                with prio(-900000 + b * 2000):
                    dstb = sgpool.tile([128, K, 512], F16, tag="dst")
                    nc.gpsimd.dma_gather(
                        dstb[:, 0:5, :], pairs[:],
                        idxw_all[:, b * 72:b * 72 + 40],
                        5 * 128, 5 * 128, 512)
                    nc.gpsimd.dma_gather(
                        dstb[:, 5:K, :], pairs[:],
                        idxw_all[:, b * 72 + 40:(b + 1) * 72],
                        4 * 128, 4 * 128, 512)
                    dst_t[b] = dstb

            def compute_blk(b):
                dstb = dst_t.pop(b)
                w_e8 = we8_c.pop(b)
                ci, lo = chunk_of_block(b)
                # the gwv multiply (DVE) interleaves with the front: it sorts
                # just after the NEXT chunk's stage-B so the DVE never parks
                # behind the whole front band; the PE/Act compute stays in the
                # late band so the front chains keep feeding the gathers
                with prio(-1009000 + (ci + 1) * 2000 + 600 + (b - lo) * 10):
                    if b + 2 < NBLK and b + 2 >= 4:
                        emit_we8(b + 2)
                    gw = gwpool.tile([128, 36, 128], F16, tag="gw")
                    dsrc = dstb[:].rearrange("p s e -> p (s e)").rearrange(
                        "p (j r q) -> p j r q", r=16, q=8)
                    gwv = gw[:].rearrange("p j (r q) -> p j r q", q=8)
                    w_in = bcast(w_e8[:].rearrange("p j (o q) -> p j o q", o=1),
                                 [128, 36, 16, 8])
                    # packed fp16 operands -> DVE 2x mode
                    nc.vector.tensor_tensor(out=gwv[:], in0=dsrc[:],
                                            in1=w_in[:], op=AL.mult)
                with prio(-890000 + b * 1000):
                    cols = blkpool.tile([128, K, 128], F16, tag="colsb")
                    # accumulating transposes: 4 taps per PSUM bank
                    for kg, pool, nk in ((0, pc0pool, 4), (1, pc1pool, 4),
                                         (2, pc2pool, 1)):
                        pcb = pool.tile([128, nk * 128], F32, tag=f"pc{kg}")
                        for kq in range(nk):
                            k = 4 * kg + kq
                            for j in range(4):
                                nc.tensor.matmul(
                                    out=pcb[:, kq * 128:(kq + 1) * 128],
                                    lhsT=gw[:, 4 * k + j, :],
                                    rhs=s_eye16[:], start=(j == 0), stop=(j == 3))
                        nc.scalar.copy(
                            out=cols[:, 4 * kg:4 * kg + nk, :],
                            in_=pcb[:].rearrange("p (k c) -> p k c", c=128))
                    # im2col GEMM: cols stationary, filter moving (FD=256),
                    # output pixel-major [pix, F]
                    po = popool.tile([128, F], F32, tag="po")
                    for k in range(K):
                        nc.tensor.matmul(
                            out=po[:],
                            lhsT=cols[:, k, :],
                            rhs=s_filt[:, k * F:(k + 1) * F],
                            start=(k == 0), stop=(k == K - 1))
                    osb = blkpool.tile([128, F], F16, tag="osb")
                    nc.scalar.copy(out=osb[:], in_=po[:])
                    nc.sync.dma_start(out=out_d[:, b, :], in_=osb[:])

            for ci in range(len(CHUNKS)):
                conv_chunk(ci)
                stageb_chunk(ci)
                if ci < 2:
                    with prio(-1009000 + ci * 2000 + 500):
                        for b in range(*CHUNKS[ci]):
                            emit_we8(b)
                for b in range(*CHUNKS[ci]):
                    emit_gather(b)
            for b in range(NBLK):
                compute_blk(b)
    nc.compile()
    return nc


def host_inputs(x, offset_kernel, offset_bias, filt_w):
    """Per-sample input maps. x [8,64,64,128] f32 etc (numpy)."""
    offk = np.ascontiguousarray(
        offset_kernel.reshape(K, C, 27).transpose(1, 0, 2).reshape(C, K * 27)
    ).astype(np.float16)
    offb = offset_bias.reshape(27, 1).astype(np.float32)
    filt_re = np.ascontiguousarray(
        filt_w.reshape(K, C, F).transpose(1, 0, 2).reshape(C, K * F)
    ).astype(np.float16)
    eye16 = np.eye(128).astype(np.float16)
    eye32 = np.eye(128).astype(np.float32)
    rep16 = np.zeros((16, 128), np.float32)
    rep16[np.arange(128) % 16, np.arange(128)] = 1.0
    consts = np.zeros((128, 348), np.float32)
    p = np.arange(128)
    yoff = p // 64
    consts[:, 0:32] = 2 * np.arange(32)[None, :] + yoff[:, None]
    consts[:, 32:41] = DY[None, :]
    consts[:, 41:50] = DX[None, :]
    consts[:, 50] = p % 64
    consts[:, 51:339] = (consts[:, 0:32, None] + DY[None, None, :]
                         + 8.0).reshape(128, 288)
    consts[:, 339:348] = consts[:, 50:51] + DX[None, :] + 8.0

    maps = []
    for b in range(x.shape[0]):
        xp = np.zeros((HP + 2, WP + 2, C), np.float32)
        xp[PADR:PADR + H, PADR:PADR + W] = x[b]
        quad = np.zeros((2, 2, 40, 40, 2, 2, C), np.float32)
        for pY in range(2):
            for pX in range(2):
                for uy in range(2):
                    for ux in range(2):
                        quad[pY, pX, :, :, uy, ux] = \
                            xp[pY + uy:pY + uy + 80:2, pX + ux:pX + ux + 80:2]
        prs = quad.reshape(NROWS, 4 * C).astype(np.float16)

        x1 = np.zeros((CONVW, CONVW, C), np.float32)
        x1[1:65, 1:65] = x[b]
        xcl = np.zeros((C, XCLM), np.float16)
        xcl[:, 67:67 + 4356] = x1.reshape(CONVW * CONVW, C).T.astype(np.float16)
        maps.append({
            "xcl": xcl, "pairs": prs, "offk": offk, "offb": offb,
            "filt": filt_re, "eye16": eye16, "eye32": eye32, "rep16": rep16,
            "consts": consts,
        })
    return maps


def host_output(res_list):
    outs = []
    for r in res_list:
        o = r["out"].astype(np.float32).reshape(128, NBLK, F)
        outs.append(np.ascontiguousarray(
            o.transpose(1, 0, 2)).reshape(H, W, F))
    return np.stack(outs)


def _get_nc():
    global _NC
    if _NC is None:
        _NC = build_nc()
    return _NC


def kernel(inputs, offset_kernel, offset_bias, filt):
    from concourse.bass_utils import run_bass_kernel_spmd
    x = np.asarray(inputs, dtype=np.float32)
    maps = host_inputs(x, np.asarray(offset_kernel, np.float32),
                       np.asarray(offset_bias, np.float32),
                       np.asarray(filt, np.float32))
    nc = _get_nc()
    res = run_bass_kernel_spmd(nc, maps, core_ids=list(range(8)))
    return host_output(res.results).astype(np.float32)
